# revision 1
# baseline (speedup 1.0000x reference)
# Gaussian-kernel ridge-regression matvec on 8 Trainium2 cores.
#
#   out_i = sum_j exp(-||x_i - y_j||^2 / g) * alpha_j
#   N=8192 queries, M=16384 train points, DIM=32, g scalar.
#
# Factorization (host prep is O(N+M), device does the O(N*M) part):
#   exp(-(x^2+y^2-2xy)/g)*a_j = exp(-x_i^2/g) * sign(a_j) * exp(s_ij),
#   s_ij = (2/g) x_i.y_j + c_j,   c_j = -y_j^2/g + ln|a_j|
# Train points are host-sorted so sign(a)>0 comes first (npos); the device
# computes s via an augmented K=33 matmul (row 32 of x~ is 1, row 32 of y~
# is c_j), exps it, and row-sums the pos and neg column ranges separately.
# Row scale exp(-x_i^2/g) is applied on host.
#
# The matmul runs in fp16 hi/lo "triple" form for near-fp32 accuracy at
# 1 cycle/row:  x.y ~= xh.yh + xh.yl + xl.yh  (xl*yl ~ 2^-22, dropped),
# accumulated in PSUM fp32.
#
# Per core (1024 rows): 8 i-tiles x 8 groups of 2048 cols; each group =
# 4 PSUM banks x 3 accumulating matmuls; ACT exp in-place on PSUM with
# accum_out giving per-row sums per segment; tiny DVE reduce/sub; one DMA.

import numpy as np

N, M, DIM, NCORES = 8192, 16384, 32, 8
NLOC = N // NCORES
ITILES = NLOC // 128
GRP = 2048
NGRP = M // GRP
KAUG = DIM + 1

_cache = {}


def _build(npos):
    import concourse.bass as bass
    import concourse.tile as tile
    from concourse import bacc, mybir

    f32 = mybir.dt.float32
    f16 = mybir.dt.float16
    Exp = mybir.ActivationFunctionType.Exp
    X = mybir.AxisListType.X

    nc = bacc.Bacc("TRN2", target_bir_lowering=False, debug=False)
    yh = nc.dram_tensor("yh", [KAUG, M], f16, kind="ExternalInput").ap()
    yl = nc.dram_tensor("yl", [KAUG, M], f16, kind="ExternalInput").ap()
    xh = nc.dram_tensor("xh", [KAUG, NLOC], f16, kind="ExternalInput").ap()
    xl = nc.dram_tensor("xl", [KAUG, NLOC], f16, kind="ExternalInput").ap()
    o = nc.dram_tensor("o", [128, ITILES], f32, kind="ExternalOutput").ap()

    segs = []
    for gi in range(NGRP):
        g0, g1 = gi * GRP, (gi + 1) * GRP
        if g0 < npos:
            segs.append((g0, min(g1, npos), True))
        if g1 > npos:
            segs.append((max(g0, npos), g1, False))
    npos_segs = sum(1 for s in segs if s[2])
    nseg = len(segs)

    with tile.TileContext(nc) as tc:
        with tc.tile_pool(name="ypool", bufs=1) as ypool, \
             tc.tile_pool(name="xpool", bufs=1) as xpool, \
             tc.tile_pool(name="psum", bufs=2, space="PSUM") as pp, \
             tc.tile_pool(name="parts", bufs=ITILES) as partp, \
             tc.tile_pool(name="small", bufs=2 * ITILES) as smallp, \
             tc.tile_pool(name="res", bufs=1) as resp:

            YCH = 4096
            yhts, ylts = [], []
            for ci in range(M // YCH):
                t = ypool.tile([KAUG, YCH], f16, tag=f"yh{ci}")
                nc.sync.dma_start(t[:], yh[:, bass.ts(ci, YCH)])
                yhts.append(t)
                t = ypool.tile([KAUG, YCH], f16, tag=f"yl{ci}")
                nc.sync.dma_start(t[:], yl[:, bass.ts(ci, YCH)])
                ylts.append(t)
            xht = xpool.tile([KAUG, NLOC], f16, tag="xh")
            nc.sync.dma_start(xht[:], xh[:])
            xlt = xpool.tile([KAUG, NLOC], f16, tag="xl")
            nc.sync.dma_start(xlt[:], xl[:])

            # Pre-touch all DMA'd tiles on the PE so real matmuls never carry
            # DMA-queue waits (walrus limits sync waits per matmul to 2, and
            # slot-recycling matmuls already need ACT+PE waits).
            dummyw = smallp.tile([KAUG, 1], f16, tag="dummyw")
            nc.vector.memset(dummyw[:], 0.0)
            dpsum = pp.tile([1, 512], f32, tag="ps")
            nc.tensor.matmul(dpsum[:, 0:1], dummyw[:], dummyw[:],
                             start=True, stop=True)
            for di, t in enumerate(yhts + ylts + [xht, xlt]):
                nc.tensor.matmul(dpsum[:, di + 1:di + 2], dummyw[:], t[:, 0:1],
                                 start=True, stop=True)

            res = resp.tile([128, ITILES], f32)

            for it in range(ITILES):
                xhw = xht[:, bass.ts(it, 128)]
                xlw = xlt[:, bass.ts(it, 128)]
                parts = partp.tile([128, nseg], f32, tag="parts")

                seg_i = 0
                for gi in range(NGRP):
                    ps = pp.tile([128, GRP], f32, tag="ps")
                    # strided memset: one element in each of the 4 banks ->
                    # this DVE op becomes the tile's first accessor and absorbs
                    # the slot-release waits (matmuls may carry only 1 wait)
                    nc.vector.memset(
                        ps[:].rearrange("p (b c) -> p b c", c=512)[:, :, 0:1], 0.0)
                    g0 = gi * GRP
                    for k in range(4):
                        j0 = g0 + k * 512
                        ci, off = j0 // YCH, j0 % YCH
                        sl = ps[:, bass.ts(k, 512)]
                        yhr = yhts[ci][:, off:off + 512]
                        ylr = ylts[ci][:, off:off + 512]
                        nc.tensor.matmul(sl, xhw, yhr, start=True, stop=False)
                        nc.tensor.matmul(sl, xhw, ylr, start=False, stop=False)
                        nc.tensor.matmul(sl, xlw, yhr, start=False, stop=True)
                    while seg_i < nseg and segs[seg_i][0] < g0 + GRP:
                        s0, s1, _pos = segs[seg_i]
                        seg = ps[:, s0 - g0: s1 - g0]
                        nc.scalar.activation(seg, seg, Exp,
                                             accum_out=parts[:, seg_i:seg_i + 1])
                        seg_i += 1

                possum = smallp.tile([128, 1], f32, tag="pos")
                negsum = smallp.tile([128, 1], f32, tag="neg")
                if npos_segs:
                    nc.vector.reduce_sum(possum[:], parts[:, 0:npos_segs], axis=X)
                else:
                    nc.vector.memset(possum[:], 0.0)
                if nseg - npos_segs:
                    nc.vector.reduce_sum(negsum[:], parts[:, npos_segs:nseg], axis=X)
                else:
                    nc.vector.memset(negsum[:], 0.0)
                nc.vector.tensor_sub(res[:, it:it + 1], possum[:], negsum[:])

            nc.sync.dma_start(o[:], res[:])

    nc.compile()
    return nc


def kernel(x, y_train, alphas, g):
    from concourse.bass_utils import run_bass_kernel_spmd

    x = np.asarray(x, dtype=np.float32)
    y_train = np.asarray(y_train, dtype=np.float32)
    a = np.asarray(alphas, dtype=np.float32).reshape(-1)
    gf = float(np.asarray(g).reshape(-1)[0])

    y2 = np.sum(y_train.astype(np.float64) ** 2, axis=1)
    with np.errstate(divide="ignore"):
        c = -y2 / gf + np.log(np.abs(a.astype(np.float64)))
    c = np.maximum(c, -1e4)

    pos = a >= 0
    order = np.concatenate([np.nonzero(pos)[0], np.nonzero(~pos)[0]])
    npos = int(pos.sum())

    ytab = np.empty((KAUG, M), dtype=np.float64)
    ytab[:DIM] = (2.0 / gf) * y_train[order].T.astype(np.float64)
    ytab[DIM] = c[order]
    yh64 = ytab.astype(np.float16).astype(np.float64)
    yhn = yh64.astype(np.float16)
    yln = (ytab - yh64).astype(np.float16)

    key = npos
    if key not in _cache:
        _cache[key] = _build(npos)
    nc = _cache[key]

    in_maps = []
    for k in range(NCORES):
        xs = x[k * NLOC:(k + 1) * NLOC]
        xtab = np.empty((KAUG, NLOC), dtype=np.float64)
        xtab[:DIM] = xs.T.astype(np.float64)
        xtab[DIM] = 1.0
        xh64 = xtab.astype(np.float16).astype(np.float64)
        in_maps.append({
            "yh": yhn, "yl": yln,
            "xh": xh64.astype(np.float16),
            "xl": (xtab - xh64).astype(np.float16),
        })

    r = run_bass_kernel_spmd(nc, in_maps, core_ids=list(range(NCORES)))

    x2 = np.sum(x.astype(np.float64) ** 2, axis=1)
    rowscale = np.exp(-x2 / gf)
    out = np.empty(N, dtype=np.float64)
    for k in range(NCORES):
        out[k * NLOC:(k + 1) * NLOC] = r.results[k]["o"].T.reshape(NLOC).astype(np.float64)
    out *= rowscale
    return out.astype(np.float32).reshape(N, 1)



# revision 3
# speedup vs baseline: 1.9023x; 1.9023x over previous
# Gaussian-kernel ridge-regression matvec on 8 Trainium2 cores.
#
#   out_i = sum_j exp(-||x_i - y_j||^2 / g) * alpha_j
#   N=8192 queries, M=16384 train points, DIM=32, g scalar.
#
# Factorization (host prep is O(N+M), device does the O(N*M) part):
#   exp(-(x^2+y^2-2xy)/g)*a_j = exp(-x_i^2/g) * sign(a_j) * exp(s_ij),
#   s_ij = (2/g) x_i.y_j + c_j,   c_j = -y_j^2/g + ln|a_j|
# Train points are host-sorted so sign(a)>0 comes first (npos); the device
# computes s via an augmented matmul, exps it, and row-sums the pos and neg
# column ranges separately. Row scale exp(-x_i^2/g) is applied on host.
#
# The fp16 hi/lo "triple" product x.y ~= xh.yh + xh.yl + xl.yh is computed
# in a SINGLE matmul pass by stacking the three terms along the contraction
# (partition) axis: K = 3*33 = 99 <= 128, with
#   lhsT = [xh; xh; xl]  (stationary, [99, 128] per row-tile)
#   rhs  = [yh; yl; yh]  (moving,     [99, 512] per PSUM bank)
# One matmul instead of three -> PE time drops 3x and the scalar engine's
# exp+accumulate (1 elem/lane/cycle @ 1.2 GHz over N*M elements) becomes the
# bottleneck. PSUM is used as two ping-pong [128, 4096] halves: PE fills one
# (8 matmuls) while ACT exp+row-accumulates the other in big 4096-wide
# instructions (fewer instructions -> less fixed overhead per element).

import numpy as np

N, M, DIM, NCORES = 8192, 16384, 32, 8
NLOC = N // NCORES
ITILES = NLOC // 128
GRP = 2048
NGRP = M // GRP
KAUG = DIM + 1
KSTK = 3 * KAUG  # 99

_cache = {}


def _build(npos):
    import concourse.bass as bass
    import concourse.tile as tile
    from concourse import bacc, mybir

    f32 = mybir.dt.float32
    f16 = mybir.dt.float16
    Exp = mybir.ActivationFunctionType.Exp
    X = mybir.AxisListType.X

    nc = bacc.Bacc("TRN2", target_bir_lowering=False, debug=False)
    ys = nc.dram_tensor("ys", [KSTK, M], f16, kind="ExternalInput").ap()
    xs = nc.dram_tensor("xs", [KSTK, NLOC], f16, kind="ExternalInput").ap()
    o = nc.dram_tensor("o", [128, ITILES], f32, kind="ExternalOutput").ap()

    segs = []
    for gi in range(NGRP):
        g0, g1 = gi * GRP, (gi + 1) * GRP
        if g0 < npos:
            segs.append((g0, min(g1, npos), True))
        if g1 > npos:
            segs.append((max(g0, npos), g1, False))
    npos_segs = sum(1 for s in segs if s[2])
    nseg = len(segs)

    with tile.TileContext(nc) as tc:
        with tc.tile_pool(name="ypool", bufs=1) as ypool, \
             tc.tile_pool(name="xpool", bufs=1) as xpool, \
             tc.tile_pool(name="psum", bufs=2, space="PSUM") as pp, \
             tc.tile_pool(name="parts", bufs=ITILES) as partp, \
             tc.tile_pool(name="small", bufs=2 * ITILES) as smallp, \
             tc.tile_pool(name="res", bufs=1) as resp:

            YCH = 4096
            ysts = []
            for ci in range(M // YCH):
                t = ypool.tile([KSTK, YCH], f16, tag=f"ys{ci}")
                nc.sync.dma_start(t[:], ys[:, bass.ts(ci, YCH)])
                ysts.append(t)
            xst = xpool.tile([KSTK, NLOC], f16, tag="xs")
            nc.sync.dma_start(xst[:], xs[:])

            # Pre-touch all DMA'd tiles on the PE so real matmuls never carry
            # DMA-queue waits (walrus limits sync waits per matmul; the
            # psum-recycling matmuls already need an ACT wait).
            dummyw = smallp.tile([KSTK, 1], f16, tag="dummyw")
            nc.vector.memset(dummyw[:], 0.0)
            dpsum = pp.tile([1, 512], f32, tag="ps")
            nc.tensor.matmul(dpsum[:, 0:1], dummyw[:], dummyw[:],
                             start=True, stop=True)
            for di, t in enumerate(ysts + [xst]):
                nc.tensor.matmul(dpsum[:, di + 1:di + 2], dummyw[:], t[:, 0:1],
                                 start=True, stop=True)

            res = resp.tile([128, ITILES], f32)

            for it in range(ITILES):
                xw = xst[:, bass.ts(it, 128)]
                parts = partp.tile([128, nseg], f32, tag="parts")

                seg_i = 0
                for gi in range(NGRP):
                    ps = pp.tile([128, GRP], f32, tag="ps")
                    # strided memset: one element in each of the 8 banks ->
                    # this DVE op becomes the tile's first accessor and absorbs
                    # the slot-release waits (matmuls may carry few waits)
                    nc.vector.memset(
                        ps[:].rearrange("p (b c) -> p b c", c=512)[:, :, 0:1], 0.0)
                    g0 = gi * GRP
                    for k in range(GRP // 512):
                        j0 = g0 + k * 512
                        ci, off = j0 // YCH, j0 % YCH
                        nc.tensor.matmul(ps[:, bass.ts(k, 512)], xw,
                                         ysts[ci][:, off:off + 512],
                                         start=True, stop=True)
                    while seg_i < nseg and segs[seg_i][0] < g0 + GRP:
                        s0, s1, _pos = segs[seg_i]
                        seg = ps[:, s0 - g0: s1 - g0]
                        nc.scalar.activation(seg, seg, Exp,
                                             accum_out=parts[:, seg_i:seg_i + 1])
                        seg_i += 1

                possum = smallp.tile([128, 1], f32, tag="pos")
                negsum = smallp.tile([128, 1], f32, tag="neg")
                if npos_segs:
                    nc.vector.reduce_sum(possum[:], parts[:, 0:npos_segs], axis=X)
                else:
                    nc.vector.memset(possum[:], 0.0)
                if nseg - npos_segs:
                    nc.vector.reduce_sum(negsum[:], parts[:, npos_segs:nseg], axis=X)
                else:
                    nc.vector.memset(negsum[:], 0.0)
                nc.vector.tensor_sub(res[:, it:it + 1], possum[:], negsum[:])

            nc.sync.dma_start(o[:], res[:])

    nc.compile()
    return nc


def kernel(x, y_train, alphas, g):
    from concourse.bass_utils import run_bass_kernel_spmd

    x = np.asarray(x, dtype=np.float32)
    y_train = np.asarray(y_train, dtype=np.float32)
    a = np.asarray(alphas, dtype=np.float32).reshape(-1)
    gf = float(np.asarray(g).reshape(-1)[0])

    y2 = np.sum(y_train.astype(np.float64) ** 2, axis=1)
    with np.errstate(divide="ignore"):
        c = -y2 / gf + np.log(np.abs(a.astype(np.float64)))
    c = np.maximum(c, -1e4)

    pos = a >= 0
    order = np.concatenate([np.nonzero(pos)[0], np.nonzero(~pos)[0]])
    npos = int(pos.sum())

    ytab = np.empty((KAUG, M), dtype=np.float64)
    ytab[:DIM] = (2.0 / gf) * y_train[order].T.astype(np.float64)
    ytab[DIM] = c[order]
    yh64 = ytab.astype(np.float16).astype(np.float64)
    yhn = yh64.astype(np.float16)
    yln = (ytab - yh64).astype(np.float16)
    ysn = np.concatenate([yhn, yln, yhn], axis=0)  # [99, M]

    key = npos
    if key not in _cache:
        _cache[key] = _build(npos)
    nc = _cache[key]

    in_maps = []
    for k in range(NCORES):
        xsl = x[k * NLOC:(k + 1) * NLOC]
        xtab = np.empty((KAUG, NLOC), dtype=np.float64)
        xtab[:DIM] = xsl.T.astype(np.float64)
        xtab[DIM] = 1.0
        xh64 = xtab.astype(np.float16).astype(np.float64)
        xhn = xh64.astype(np.float16)
        xln = (xtab - xh64).astype(np.float16)
        in_maps.append({
            "ys": ysn,
            "xs": np.concatenate([xhn, xhn, xln], axis=0),  # [99, NLOC]
        })

    r = run_bass_kernel_spmd(nc, in_maps, core_ids=list(range(NCORES)))

    x2 = np.sum(x.astype(np.float64) ** 2, axis=1)
    rowscale = np.exp(-x2 / gf)
    out = np.empty(N, dtype=np.float64)
    for k in range(NCORES):
        out[k * NLOC:(k + 1) * NLOC] = r.results[k]["o"].T.reshape(NLOC).astype(np.float64)
    out *= rowscale
    return out.astype(np.float32).reshape(N, 1)
